# revision 79
# baseline (speedup 1.0000x reference)
"""Trainium2 Bass kernel for nn_CutBayesFlow.

Data-parallel over batch N=8192 across 8 NeuronCores (1024 samples/core).
Flow params and data summary stats are replicated; each core returns its
per-sample losses [128, 8]; the host averages to the scalar.

Spline gathers are computed as bf16 masked products + a shared 4-level
bf16 tree reduction (two monotone masks sA/sB against the bin-edge
cumsum) instead of per-bin suffix scans. x is stored pre-shifted by
+TAIL (fold into host biases) so the combine drops the -TAIL ops.

Self-contained: all shapes hardcoded, no sibling imports.
"""
import sys
import numpy as np

for _p in ("/opt/trn_rl_repo",):
    if _p not in sys.path:
        sys.path.insert(0, _p)

import ml_dtypes
import concourse.bass as bass
import concourse.bacc as bacc
import concourse.tile as tile
import concourse.mybir as mybir
from contextlib import ExitStack
from concourse.bass_utils import run_bass_kernel_spmd

F32 = mybir.dt.float32
BF16 = mybir.dt.bfloat16
FP8 = mybir.dt.float8e4
SCALE_WO = 64.0   # keep fp8 W_out out of the subnormal range
AF = mybir.ActivationFunctionType
ALU = mybir.AluOpType

# Pin all ScalarE activations to the one table that holds every function we
# use (ln, exp, relu, identity, copy, square) so the act-table-load pass
# never has to switch tables mid-kernel. Indexes are preserved; tables other
# than the chosen one are emptied so the pass cannot pick them.
_ONE_TABLE = "natural_log_exp_and_others"
_orig_get_act_tables = bacc.get_activation_tables


def _pinned_act_tables(arch):
    tabs = _orig_get_act_tables(arch)
    return {name: (fns if name == _ONE_TABLE else set())
            for name, fns in tabs.items()}


bacc.get_activation_tables = _pinned_act_tables

# ---- problem constants (hardcoded) ----
F = 64; HID = 256; P_ETA = 32; NB = 16; MULT = 3 * NB - 1   # 47
NL = 8; TAIL = 10.0
MIN_BW = 1e-8; MIN_D = 1e-8
N_BATCH = 8192; M_DATA = 256
NCORES = 8
NPC = N_BATCH // NCORES          # 1024 samples per core
P = 128                          # partitions
NTILES = NPC // P                # 8 batch-tiles per core
CHUNKS = 4                       # chunks per core
TPC = 2                          # batch-tiles per chunk
NBC = TPC * P                    # 256 samples per chunk

A_CONST = 1.0 - MIN_BW * NB
INV2TA = 1.0 / (2.0 * TAIL * A_CONST)
TWO_TA = 2.0 * TAIL * A_CONST
LOG_EPS = float(np.log(1e-10))           # -23.02585...
C32 = float(0.5 * F * np.log(2.0 * np.pi))
BOUND_D = 1.0 - MIN_D
_SCAN_ON_GPSIMD = False
_SCAN_ENG = None


def _build_program(nl=NL, debug_out=False):
    nc = bacc.Bacc("TRN2", target_bir_lowering=False, debug=False)

    # ---------------- DRAM I/O ----------------
    d_xs0 = nc.dram_tensor("xs0", [P, NTILES, F], F32, kind="ExternalInput")
    d_etaT = nc.dram_tensor("etaT", [P_ETA, NPC], BF16, kind="ExternalInput")
    d_ident = nc.dram_tensor("ident", [P, P], F32, kind="ExternalInput")
    d_identb = nc.dram_tensor("identb", [P, P], BF16, kind="ExternalInput")
    d_dmw = nc.dram_tensor("dmw", [P, 66], F32, kind="ExternalInput")
    d_wtsr = nc.dram_tensor("wtsr", [P, F], F32, kind="ExternalInput")
    d_ones = nc.dram_tensor("ones1", [1, P], BF16, kind="ExternalInput")
    d_wi = nc.dram_tensor("wi_all", [NL, F, HID], BF16, kind="ExternalInput")
    d_wc = nc.dram_tensor("wc_all", [NL, P_ETA, HID], BF16, kind="ExternalInput")
    d_b01 = nc.dram_tensor("b01_all", [NL, P, 2], F32, kind="ExternalInput")
    d_wb = nc.dram_tensor("wb_all", [NL, P, 2, 2, 2, 2, P], BF16,
                          kind="ExternalInput")
    d_bb = nc.dram_tensor("bb_all", [NL, P, 8], F32, kind="ExternalInput")
    d_wo = nc.dram_tensor("wo_all", [NL, P, 2, 3008], FP8, kind="ExternalInput")
    d_bo = nc.dram_tensor("bo_all", [NL, 1, 3008], BF16, kind="ExternalInput")
    d_loss = nc.dram_tensor("loss_out", [P, NTILES], F32, kind="ExternalOutput")
    d_dbg = None
    if debug_out:
        d_dbg = nc.dram_tensor("dbg_out", [P, 2 * NTILES, F], F32,
                               kind="ExternalOutput")

    bctx = ExitStack()
    with tile.TileContext(nc) as tc:
        with bctx:
            _emit(bctx, tc, nc, d_xs0, d_etaT, d_ident, d_identb, d_dmw, d_wtsr,
                  d_ones, d_wi, d_wc, d_b01, d_wb, d_bb, d_wo, d_bo, d_loss,
                  nl=nl, d_dbg=d_dbg)
    nc.compile()
    return nc


def _fv(buf, off, dims):
    """View of a tile at element offset `off` with [stride, count] dims."""
    a = buf[:]
    return bass.AP(a.tensor, a.offset + off, [list(a.ap[0])] + dims)


def _emit(ctx, tc, nc, d_xs0, d_etaT, d_ident, d_identb, d_dmw, d_wtsr, d_ones,
          d_wi, d_wc, d_b01, d_wb, d_bb, d_wo, d_bo, d_loss, nl=NL, d_dbg=None):
    CB = TPC * F * NB             # 2048 per-partition bin elems per chunk
    Z = TPC * F                   # 128 combine width per chunk
    NSEG = TPC * F                # 128 bin segments per chunk

    global _SCAN_ENG
    _SCAN_ENG = nc.gpsimd if _SCAN_ON_GPSIMD else nc.vector
    pconst = ctx.enter_context(tc.tile_pool(name="pconst", bufs=1))
    pw = ctx.enter_context(tc.tile_pool(name="pw", bufs=2))
    pmlp = ctx.enter_context(tc.tile_pool(name="pmlp", bufs=2))
    pcmb = ctx.enter_context(tc.tile_pool(name="pcmb", bufs=1))
    ps_mm = ctx.enter_context(tc.tile_pool(name="ps_mm", bufs=2, space="PSUM"))
    ps_uw = ctx.enter_context(tc.tile_pool(name="ps_uw", bufs=1, space="PSUM"))
    ps_uh = ctx.enter_context(tc.tile_pool(name="ps_uh", bufs=1, space="PSUM"))
    ps_ud = ctx.enter_context(tc.tile_pool(name="ps_ud", bufs=1, space="PSUM"))

    # ---------------- persistent tiles ----------------
    xs = pconst.tile([P, NTILES, F], F32, tag="xs")     # holds x + TAIL
    nc.sync.dma_start(xs[:], d_xs0[:])
    etaT = pconst.tile([P_ETA, NPC], BF16, tag="etaT")
    nc.sync.dma_start(etaT[:], d_etaT[:])
    ident = pconst.tile([P, P], F32, tag="ident")
    nc.sync.dma_start(ident[:], d_ident[:])
    identb = pconst.tile([P, P], BF16, tag="identb")
    nc.sync.dma_start(identb[:], d_identb[:])
    dmw = pconst.tile([P, 66], F32, tag="dmw")
    nc.sync.dma_start(dmw[:], d_dmw[:])
    wtsr = pconst.tile([P, F], F32, tag="wtsr")
    nc.sync.dma_start(wtsr[:], d_wtsr[:])
    ones1 = pconst.tile([1, P], BF16, tag="ones1")
    nc.sync.dma_start(ones1[:], d_ones[:])

    segm = pconst.tile([P, CB], BF16, tag="segm")     # 1 except 0 at k=0 of 16
    nc.vector.memset(segm[:], 1.0)
    nc.vector.memset(segm[:, 0::NB], 0.0)
    # per-bin sources packed in one mega buffer [ew | eh | dbR | dbL], each
    # lane flat [P, 2048] bf16, logical layout (t, f, k). Double-buffered so
    # chunk c+1's ScalarE evacs don't stall on chunk c's DVE reads.
    binsrc2 = []
    for i in range(2):
        bsi = pconst.tile([P, 4 * CB], BF16, tag=f"bs{i}", name=f"bsi{i}")
        nc.vector.memset(bsi[:, 2 * CB::NB], BOUND_D)        # dbR slot0 = d[0]
        nc.vector.memset(bsi[:, 3 * CB + NB - 1::NB], BOUND_D)  # dbL slot15
        binsrc2.append(bsi)
    E_t2 = [pconst.tile([P, CB], F32, tag=f"E_t{i}", name=f"E_t{i}")
            for i in range(2)]                        # incl cumsum of ew (exact)
    sB = pconst.tile([P, CB], BF16, tag="sB")         # [X >= E_k]
    ind = pconst.tile([P, CB], BF16, tag="ind")       # one-hot bin indicator
    # products scratch: 4 exact-selection lanes [ind*ew, ind*eh, ind*dbR,
    # ind*dbL] (bf16 tree is exact: zeros + one value) + tree lanes 4/5 for
    # the cumsum gPp (sB*eh) and total Sh (copy of eh); lane 6 (sB*ew) is
    # reduced with f32 accumulation (exact) because u = Xt - gP cancels.
    scrP = pconst.tile([P, 6 * CB], BF16, tag="scrP")
    scrA = pconst.tile([P, 4 * CB // 2], BF16, tag="scrA")
    res6 = pconst.tile([P, 4 * NSEG], F32, tag="res6")
    gPx = pconst.tile([P, 2 * NSEG], F32, tag="gPx")
    ld_t = pconst.tile([P, NTILES], F32, tag="ld_t")
    ldacc = pconst.tile([P, NTILES, F], F32, tag="ldacc")
    nc.vector.memset(ldacc[:], 0.0)
    zz_t = pconst.tile([P, NTILES], F32, tag="zz_t")
    negT = pconst.tile([P, 1], F32, tag="negT")
    nc.vector.memset(negT[:], -TAIL)

    def seg3(buf, off, n=NB):
        # [P, NSEG, n] view of a flat [P, CB]-ish buffer at element offset
        return _fv(buf, off, [[NB, NSEG], [1, n]])

    def colNB(buf, c):
        # [P, TPC, F] view picking bin-column c
        return _fv(buf, c, [[F * NB, TPC], [NB, F]])

    def v3(buf, off=0):
        # [P, TPC, F] view of a flat [P, Z] f32 tile
        return _fv(buf, off, [[F, TPC], [1, F]])

    pending = {}

    def flush_pending():
        if not pending:
            return
        inner, vv, ldv = pending.pop("inner"), pending.pop("vv"), pending.pop("ldv")
        lni = pcmb.tile([P, Z], F32, tag="lni", name="lni")
        lnv = pcmb.tile([P, Z], F32, tag="lnv", name="lnv")
        nc.scalar.activation(lni[:], inner[:], AF.Ln)
        nc.scalar.activation(lnv[:], vv[:], AF.Ln)
        ldf = pcmb.tile([P, Z], F32, tag="ldf", name="ldf")
        nc.vector.scalar_tensor_tensor(ldf[:], lnv[:], 2.0, lni[:], ALU.mult,
                                       ALU.add)
        nc.gpsimd.tensor_tensor(ldv, ldv, ldf[:].rearrange("p (t f) -> p t f", t=TPC),
                                ALU.add)

    # ---------------- layer loop ----------------
    for l in range(nl):
        wi = pw.tile([F, HID], BF16, tag="wi")
        nc.sync.dma_start(wi[:], d_wi[l])
        wc = pw.tile([P_ETA, HID], BF16, tag="wc")
        nc.sync.dma_start(wc[:], d_wc[l])
        b01 = pw.tile([P, 2], F32, tag="b01")
        nc.sync.dma_start(b01[:], d_b01[l])
        wb = pw.tile([P, 2, 2, 2, 2, P], BF16, tag="wb")
        nc.sync.dma_start(wb[:], d_wb[l])
        bb = pw.tile([P, 8], F32, tag="bb")
        nc.sync.dma_start(bb[:], d_bb[l])
        wo = pw.tile([P, 2, 3008], FP8, tag="wo")
        nc.sync.dma_start(wo[:], d_wo[l])
        bo = pw.tile([1, 3008], BF16, tag="bo")
        nc.sync.dma_start(bo[:], d_bo[l])

        for c in range(CHUNKS):
            bs = binsrc2[c % 2]
            # ---- x transpose -> xT bf16 [F, NBC] (xT carries x + TAIL) ----
            xT = pmlp.tile([F, NBC], BF16, tag="xT")
            for t in range(TPC):
                pt = ps_mm.tile([F, P], F32, tag="mm")
                nc.tensor.matmul(pt[:], xs[:, TPC * c + t, :], ident[:],
                                 is_transpose=True)
                nc.scalar.copy(xT[:, t * P:(t + 1) * P], pt[:])
                if l == 0:
                    scr = pcmb.tile([P, F], F32, tag="sqscr")
                    nc.scalar.activation(scr[:], xs[:, TPC * c + t, :], AF.Square,
                                         bias=negT[:],
                                         accum_out=zz_t[:, TPC * c + t:TPC * c + t + 1])
            # ---- h = x@WiT + eta@WcT (+bias; TAIL-shift folded into b01).
            # No DVE ops anywhere in the MLP: the DVE instruction queue stays
            # pure-spline, so chunk c+1's MLP pipelines under chunk c's DVE.
            h_sb = pmlp.tile([P, 2, HID], BF16, tag="h")
            r = pmlp.tile([P, 2, HID], BF16, tag="r")
            eta_sl = etaT[:, c * NBC:(c + 1) * NBC]
            phs = []
            for mc in range(2):
                ph = ps_mm.tile([P, NBC], F32, tag="mm")
                nc.tensor.matmul(ph[:], wi[:, mc * P:(mc + 1) * P], xT[:],
                                 start=True, stop=False)
                nc.tensor.matmul(ph[:], wc[:, mc * P:(mc + 1) * P], eta_sl,
                                 start=False, stop=True)
                phs.append(ph)
            for mc in range(2):
                nc.scalar.activation(h_sb[:, mc], phs[mc][:], AF.Identity,
                                     bias=b01[:, mc:mc + 1])
            for mc in range(2):
                nc.scalar.activation(r[:, mc], phs[mc][:], AF.Relu,
                                     bias=b01[:, mc:mc + 1])
            # ---- residual blocks (residual added via identity-matmul into
            # PSUM so h_new/r_nxt come out of ScalarE evacs, not DVE) ----
            for blk in range(2):
                r1 = pmlp.tile([P, 2, HID], BF16, tag="r1")
                for mc in range(2):
                    pt1 = ps_mm.tile([P, NBC], F32, tag="mm")
                    for kc in range(2):
                        nc.tensor.matmul(pt1[:], wb[:, blk, 0, kc, mc, :],
                                         r[:, kc], start=(kc == 0), stop=(kc == 1))
                    j = blk * 4 + 0 * 2 + mc
                    nc.scalar.activation(r1[:, mc], pt1[:], AF.Relu,
                                         bias=bb[:, j:j + 1])
                h_new = pmlp.tile([P, 2, HID], BF16 if blk == 0 else FP8,
                                  tag="h" if blk == 0 else "h8", name="h_new")
                r_nxt = pmlp.tile([P, 2, HID], BF16, tag="r")
                pt2s = []
                for mc in range(2):
                    pt2 = ps_mm.tile([P, NBC], F32, tag="mm")
                    nc.tensor.matmul(pt2[:], identb[:], h_sb[:, mc],
                                     start=True, stop=False)
                    for kc in range(2):
                        nc.tensor.matmul(pt2[:], wb[:, blk, 1, kc, mc, :],
                                         r1[:, kc], start=False, stop=(kc == 1))
                    pt2s.append(pt2)
                for mc in range(2):
                    j = blk * 4 + 1 * 2 + mc
                    nc.scalar.activation(h_new[:, mc], pt2s[mc][:], AF.Identity,
                                         bias=bb[:, j:j + 1])
                if blk == 0:
                    for mc in range(2):
                        j = blk * 4 + 1 * 2 + mc
                        nc.scalar.activation(r_nxt[:, mc], pt2s[mc][:], AF.Relu,
                                             bias=bb[:, j:j + 1])
                h_sb = h_new
                r = r_nxt
            # ---- out matmuls + Exp/Softplus evacs per batch-tile ----
            for t in range(TPC):
                p_uw = ps_uw.tile([P, 1024], F32, tag="puw")
                p_uh = ps_uh.tile([P, 1024], F32, tag="puh")
                p_ud = ps_ud.tile([P, 960], F32, tag="pud")
                for (ps_t, cstart, total) in ((p_uw, 0, 1024), (p_uh, 1024, 1024),
                                              (p_ud, 2048, 960)):
                    n0 = 0
                    while n0 < total:
                        nsz = min(512, total - n0)
                        sl = ps_t[:, n0:n0 + nsz]
                        nc.tensor.matmul(sl, ones1[:],
                                         bo[:, cstart + n0:cstart + n0 + nsz],
                                         start=True, stop=False)
                        nc.tensor.matmul(sl, h_sb[:, :, t * P:(t + 1) * P],
                                         wo[:, :, cstart + n0:cstart + n0 + nsz],
                                         start=False, stop=True,
                                         perf_mode=mybir.MatmulPerfMode.DoubleRow)
                        n0 += nsz
                nc.scalar.activation(_fv(bs, 0 * CB + t * F * NB, [[1, 1024]]),
                                     p_uw[:], AF.Exp, scale=1.0 / 16.0 / SCALE_WO)
                nc.scalar.activation(_fv(bs, 1 * CB + t * F * NB, [[1, 1024]]),
                                     p_uh[:], AF.Exp, scale=1.0 / 16.0 / SCALE_WO)
                spe = pcmb.tile([P, 960], BF16, tag=f"spe{t}", name=f"spe{t}")
                nc.scalar.activation(spe[:], p_ud[:], AF.Exp, scale=1.0 / SCALE_WO)
                nc.scalar.activation(_fv(bs, 2 * CB + t * F * NB + 1,
                                         [[NB, F], [1, NB - 1]]),
                                     spe[:], AF.Ln, bias=1.0)
                nc.scalar.activation(_fv(bs, 3 * CB + t * F * NB,
                                         [[NB, F], [1, NB - 1]]),
                                     spe[:], AF.Ln, bias=1.0)
            flush_pending()
            # ---- spline section (cumsum scan on GpSimd, rest on DVE) ----
            E_t = E_t2[c % 2]
            _SCAN_ENG.tensor_tensor_scan(E_t[:], segm[:], _fv(bs, 0, [[1, CB]]),
                                         0.0, ALU.mult, ALU.add)
            # Xt = (x + T) * S_w * INV2TA  (xs holds x+T)
            Xt = pcmb.tile([P, Z], F32, tag="Xt")
            xs_ch = xs[:, TPC * c:TPC * (c + 1), :]
            nc.vector.scalar_tensor_tensor(v3(Xt), xs_ch, INV2TA, colNB(E_t, NB - 1),
                                           ALU.mult, ALU.mult)
            # mask vs exact f32 edges: sB_k = [X >= E_k] (k=0..15)
            Xb16 = _fv(Xt, 0, [[1, Z], [0, NB]])
            nc.vector.tensor_tensor(seg3(sB, 0), Xb16, seg3(E_t, 0), ALU.is_ge)
            # one-hot indicator: ind_k = sB_{k-1} - sB_k (sB_{-1} := 1)
            nc.vector.tensor_tensor(seg3(ind, 1, NB - 1), seg3(sB, 0, NB - 1),
                                    seg3(sB, 1, NB - 1), ALU.subtract)
            nc.vector.tensor_scalar(_fv(ind, 0, [[NB, NSEG]]),
                                    _fv(sB, 0, [[NB, NSEG]]),
                                    -1.0, 1.0, ALU.mult, ALU.add)
            # products: lanes 0-3 ind-masked, lane 4 sB*ew, lane 5 sB*eh
            # (flat [P, 2048] bf16, 2x mode)
            for q, (soff, msk) in enumerate(((0, ind), (CB, ind), (2 * CB, ind),
                                             (3 * CB, ind), (0, sB), (CB, sB))):
                nc.vector.tensor_tensor(_fv(scrP, q * CB, [[1, CB]]),
                                        _fv(bs, soff, [[1, CB]]),
                                        msk[:], ALU.mult)
            # 4-level exact bf16 tree over lanes 0-3 -> res6 [P, 4*NSEG] f32
            NL1 = 4 * NSEG  # 512 segments across selection lanes
            nc.vector.tensor_tensor(_fv(scrA, 0, [[8, NL1], [1, 8]]),
                                    _fv(scrP, 0, [[NB, NL1], [1, 8]]),
                                    _fv(scrP, 8, [[NB, NL1], [1, 8]]), ALU.add)
            nc.vector.tensor_tensor(_fv(scrP, 0, [[4, NL1], [1, 4]]),
                                    _fv(scrA, 0, [[8, NL1], [1, 4]]),
                                    _fv(scrA, 4, [[8, NL1], [1, 4]]), ALU.add)
            nc.vector.tensor_tensor(_fv(scrA, 0, [[2, NL1], [1, 2]]),
                                    _fv(scrP, 0, [[4, NL1], [1, 2]]),
                                    _fv(scrP, 2, [[4, NL1], [1, 2]]), ALU.add)
            nc.vector.tensor_tensor(_fv(res6, 0, [[1, NL1]]),
                                    _fv(scrA, 0, [[2, NL1]]),
                                    _fv(scrA, 1, [[2, NL1]]), ALU.add)
            # lanes 4/5 (sB*ew, sB*eh): one f32-accumulated segment reduce ->
            # exact exclusive cumsums at idx (consistent with the f32 scan)
            nc.vector.tensor_reduce(_fv(gPx, 0, [[1, 2 * NSEG]]),
                                    _fv(scrP, 4 * CB, [[NB, 2 * NSEG], [1, NB]]),
                                    mybir.AxisListType.X, ALU.add)
            # Sh = sum_k eh_k
            Sh_t = pcmb.tile([P, Z], F32, tag="Sh_t", name="Sh_t")
            nc.vector.tensor_reduce(v3(Sh_t), _fv(bs, CB, [[NB, NSEG], [1, NB]]),
                                    mybir.AxisListType.X, ALU.add)
            # ---- combine ([P, Z]) ----
            Qe_s = _fv(res6, 0 * Z, [[1, Z]]); Qh_s = _fv(res6, 1 * Z, [[1, Z]])
            d0 = _fv(res6, 2 * Z, [[1, Z]]); d1 = _fv(res6, 3 * Z, [[1, Z]])
            gPp_v = _fv(gPx, NSEG, [[1, Z]])
            Sh_v = Sh_t[:]
            cZ = lambda tg: pcmb.tile([P, Z], F32, tag=tg, name=tg)
            TT = nc.vector.tensor_tensor
            TS = nc.vector.tensor_scalar
            STT = nc.vector.scalar_tensor_tensor
            gTT = nc.gpsimd.tensor_tensor
            Qe = cZ("Qe"); nc.vector.tensor_scalar_max(Qe[:], Qe_s, 1e-6)
            u = cZ("u"); TT(u[:], Xt[:], _fv(gPx, 0, [[1, Z]]), ALU.subtract)
            iQe = cZ("iQe"); nc.vector.reciprocal_approx_fast(iQe[:], Qe[:])
            tv = cZ("tv"); TT(tv[:], u[:], iQe[:], ALU.mult)
            om = cZ("om"); TS(om[:], tv[:], -1.0, 1.0, ALU.mult, ALU.add)
            ttv = cZ("ttv"); TT(ttv[:], tv[:], om[:], ALU.mult)
            rSh = cZ("rSh"); nc.vector.reciprocal_approx_fast(rSh[:], Sh_v)
            rho = cZ("rho"); nc.vector.tensor_scalar_mul(rho[:], rSh[:], TWO_TA)
            q_ = cZ("q"); TT(q_[:], Qh_s, iQe[:], ALU.mult)
            sr = cZ("sr"); TT(v3(sr), colNB(E_t, NB - 1), v3(rSh), ALU.mult)
            delta = cZ("delta"); TT(delta[:], q_[:], sr[:], ALU.mult)
            a1 = cZ("a1"); gTT(a1[:], d0, d1, ALU.add)
            a2 = cZ("a2"); STT(a2[:], delta[:], -2.0, a1[:], ALU.mult, ALU.add)
            den = cZ("den"); TT(den[:], a2[:], ttv[:], ALU.mult)
            TT(den[:], den[:], delta[:], ALU.add)
            idn = cZ("idn"); nc.vector.reciprocal_approx_fast(idn[:], den[:])
            t2 = cZ("t2"); gTT(t2[:], tv[:], tv[:], ALU.mult)
            om2 = cZ("om2"); gTT(om2[:], om[:], om[:], ALU.mult)
            b1 = cZ("b1"); gTT(b1[:], d1, t2[:], ALU.mult)
            b2 = cZ("b2"); gTT(b2[:], delta[:], ttv[:], ALU.mult)
            b3 = cZ("b3"); gTT(b3[:], d0, om2[:], ALU.mult)
            inner = cZ("inner"); STT(inner[:], b2[:], 2.0, b1[:], ALU.mult, ALU.add)
            gTT(inner[:], inner[:], b3[:], ALU.add)
            n1 = cZ("n1"); gTT(n1[:], delta[:], t2[:], ALU.mult)
            n2 = cZ("n2"); gTT(n2[:], d0, ttv[:], ALU.mult)
            numy = cZ("numy"); gTT(numy[:], n1[:], n2[:], ALU.add)
            ich = cZ("ich"); TT(ich[:], rho[:], gPp_v, ALU.mult)
            ih = cZ("ih"); TT(ih[:], rho[:], Qh_s, ALU.mult)
            g_ = cZ("g_"); TT(g_[:], numy[:], idn[:], ALU.mult)
            yv = cZ("yv"); TT(yv[:], ih[:], g_[:], ALU.mult)
            # write y+TAIL directly into xs via the final add (a compute op --
            # must not be copy-propagated into the next layer's transpose)
            nc.gpsimd.tensor_tensor(xs_ch, v3(yv), v3(ich), ALU.add)
            vv = cZ("vv"); gTT(vv[:], delta[:], idn[:], ALU.mult)
            pending["inner"] = inner
            pending["vv"] = vv
            pending["ldv"] = ldacc[:, TPC * c:TPC * (c + 1), :]

    flush_pending()
    nc.vector.tensor_reduce(ld_t[:], ldacc[:], mybir.AxisListType.X, ALU.add)

    if d_dbg is not None:
        # dump xs (x+TAIL after `nl` layers) and per-sample flow ld
        nc.sync.dma_start(d_dbg[:, 0:NTILES, :], xs[:])
        scrd = pcmb.tile([P, NTILES, F], F32, tag="scrd")
        nc.vector.memset(scrd[:], 0.0)
        nc.vector.tensor_copy(scrd[:, :, 0], ld_t[:])
        nc.vector.tensor_copy(scrd[:, :, 1], zz_t[:])
        nc.sync.dma_start(d_dbg[:, NTILES:2 * NTILES, :], scrd[:])

    # ---------------- tail: stick-breaking + loss ----------------
    SPN = NTILES * (F + 1)        # 520

    def tb(tag, n):
        return pconst.tile([P, n], F32, tag=tag, name=tag)

    # stored x+T after 8 layers is feature-reversed (parity of last layer=1);
    # un-reverse and un-shift (-TAIL) in one DVE op.
    xr_rev = bass.AP(xs[:].tensor, xs[:].offset + (F - 1),
                     [list(xs[:].ap[0]), [F, NTILES], [-1, F]])
    xnat = pconst.tile([P, NTILES, F], F32, tag="xnat")
    nc.vector.tensor_scalar(xnat[:], xr_rev, 1.0, -TAIL, ALU.mult, ALU.add)
    xr = xnat[:]
    t_spp = tb("t_spp", SPN)
    spp = _fv(t_spp, 0, [[F + 1, NTILES], [1, F + 1]])
    nc.vector.memset(spp[:, :, 0], 0.0)
    t_spn = tb("t_spn", NTILES * F)
    spn = _fv(t_spn, 0, [[F, NTILES], [1, F]])
    t_e1 = tb("t_e1", NTILES * F)
    e1 = _fv(t_e1, 0, [[F, NTILES], [1, F]])
    nc.scalar.activation(e1, xr, AF.Exp)
    nc.scalar.activation(spp[:, :, 1:F + 1], e1, AF.Ln, bias=1.0)
    nc.scalar.activation(e1, xr, AF.Exp, scale=-1.0)
    nc.scalar.activation(spn, e1, AF.Ln, bias=1.0)
    t_seg = tb("t_seg", SPN)
    nc.vector.memset(t_seg[:], 1.0)
    nc.vector.memset(t_seg[:, 0::F + 1], 0.0)
    t_cum = tb("t_cum", SPN)
    nc.vector.tensor_tensor_scan(t_cum[:], t_seg[:], t_spp[:], 0.0,
                                 ALU.mult, ALU.add)
    cum3 = _fv(t_cum, 0, [[F + 1, NTILES], [1, F + 1]])
    t_ltn = tb("t_ltn", SPN)
    ltn = _fv(t_ltn, 0, [[F + 1, NTILES], [1, F + 1]])
    nc.vector.tensor_tensor(ltn[:, :, 0:F], spn, cum3[:, :, 0:F], ALU.add)
    nc.vector.tensor_copy(ltn[:, :, F], cum3[:, :, F])
    t_mn = tb("t_mn", SPN)
    mn = _fv(t_mn, 0, [[F + 1, NTILES], [1, F + 1]])
    nc.vector.tensor_scalar_min(mn, ltn, -LOG_EPS)
    lpn = pconst.tile([P, NTILES], F32, tag="lpn")
    nc.vector.tensor_reduce(lpn[:], mn, mybir.AxisListType.X, ALU.add)
    t_th = tb("t_th", SPN)
    th = _fv(t_th, 0, [[F + 1, NTILES], [1, F + 1]])
    nc.scalar.activation(th, ltn, AF.Exp, scale=-1.0)
    t_q1 = tb("t_q1", SPN)
    q1 = _fv(t_q1, 0, [[F + 1, NTILES], [1, F + 1]])
    dmb = dmw[:, 0:F + 1].unsqueeze(1).broadcast_to([P, NTILES, F + 1])
    nc.vector.scalar_tensor_tensor(q1, dmb, -2.0, th, ALU.mult, ALU.add)
    nc.vector.tensor_tensor(q1, q1, th, ALU.mult)
    qs = pconst.tile([P, NTILES], F32, tag="qs")
    nc.vector.tensor_reduce(qs[:], q1, mybir.AxisListType.X, ALU.add)
    t_w1 = tb("t_w1", NTILES * F)
    w1 = _fv(t_w1, 0, [[F, NTILES], [1, F]])
    wtb = wtsr[:].unsqueeze(1).broadcast_to([P, NTILES, F])
    nc.vector.tensor_tensor(w1, wtb, spp[:, :, 1:F + 1], ALU.mult)
    nc.vector.tensor_tensor(w1, w1, spn, ALU.add)
    sbn = pconst.tile([P, NTILES], F32, tag="sbn")
    nc.vector.tensor_reduce(sbn[:], w1, mybir.AxisListType.X, ALU.add)
    # loss = -0.5 zz - C32 - ld + sbn + lpn + 0.5 qs + (0.5 d_sq)
    o1 = pconst.tile([P, NTILES], F32, tag="o1")
    nc.vector.scalar_tensor_tensor(o1[:], zz_t[:], -0.5, ld_t[:], ALU.mult,
                                   ALU.subtract)
    o2 = pconst.tile([P, NTILES], F32, tag="o2")
    nc.vector.scalar_tensor_tensor(o2[:], qs[:], 0.5, sbn[:], ALU.mult, ALU.add)
    nc.vector.tensor_tensor(o1[:], o1[:], o2[:], ALU.add)
    nc.vector.tensor_tensor(o1[:], o1[:], lpn[:], ALU.add)
    loss = pconst.tile([P, NTILES], F32, tag="loss")
    nc.vector.tensor_scalar_add(loss[:], o1[:], dmw[:, 65:66])
    nc.sync.dma_start(d_loss[:], loss[:])


# ---------------- host side ----------------
_CACHE = {}


def _host_prep(z, eta_batch, data_D2, W_in, b_in, W_ctx, b_ctx, W_blk, b_blk,
               W_out, b_out):
    f32 = np.float32
    bf = ml_dtypes.bfloat16
    in_deg = np.arange(1, F + 1)
    hid_deg = np.arange(HID) % (F - 1) + 1
    m_in = (hid_deg[:, None] >= in_deg[None, :]).astype(f32)
    m_hh = (hid_deg[:, None] >= hid_deg[None, :]).astype(f32)
    out_deg = np.repeat(in_deg, MULT)
    m_out = (out_deg[:, None] > hid_deg[None, :]).astype(f32)

    Wi = (W_in * m_in[None]).astype(f32)
    Wb = (W_blk * m_hh[None, None, None]).astype(f32)
    Wo_m = (W_out * m_out[None]).astype(f32)
    # fold the per-layer feature flip into host permutations (parity trick):
    # stored feature j corresponds to logical feature lj = F-1-j on odd layers
    Wi_eff = np.empty_like(Wi)
    Wo_p = np.empty_like(Wo_m)
    bo_p = np.empty_like(b_out)
    for l in range(NL):
        par = l % 2
        Wi_eff[l] = Wi[l][:, ::-1] if par else Wi[l]
        perm = np.empty(3008, dtype=np.int64)
        for j in range(F):
            lj = (F - 1 - j) if par else j
            perm[j * 16:(j + 1) * 16] = lj * 47 + np.arange(16)
            perm[1024 + j * 16:1024 + (j + 1) * 16] = lj * 47 + 16 + np.arange(16)
            perm[2048 + j * 15:2048 + (j + 1) * 15] = lj * 47 + 32 + np.arange(15)
        Wo_p[l] = Wo_m[l][perm, :]
        bo_p[l] = b_out[l][perm]
    bo_p = bo_p.astype(f32)

    wi_all = np.ascontiguousarray(Wi_eff.transpose(0, 2, 1)).astype(bf)  # [L,64,256]
    wc_all = np.ascontiguousarray(W_ctx.transpose(0, 2, 1)).astype(bf)   # [L,32,256]
    # xs carries x+TAIL: fold -TAIL * rowsum(Wi_eff) into the bias
    wi_rowsum = np.asarray(wi_all, dtype=f32).sum(axis=1)                # [L, HID]
    b01 = (b_in + b_ctx - TAIL * wi_rowsum).astype(f32)
    b01_all = np.ascontiguousarray(
        b01.reshape(NL, 2, P).transpose(0, 2, 1)).astype(f32)
    # wb_all[l, p, blk, sub, kc, mc, m] = (Wb[l,blk,sub]*m_hh).T[kc*128+p, mc*128+m]
    WbT = Wb.transpose(0, 1, 2, 4, 3)          # [L, blk, sub, in(256), out(256)]
    wb6 = WbT.reshape(NL, 2, 2, 2, P, 2, P)    # [L, blk, sub, kc, p, mc, m]
    wb_all = np.ascontiguousarray(
        wb6.transpose(0, 4, 1, 2, 3, 5, 6)).astype(bf)  # [L, p, blk, sub, kc, mc, m]
    bb_all = np.ascontiguousarray(
        b_blk.reshape(NL, 2, 2, 2, P).transpose(0, 4, 1, 2, 3).reshape(NL, P, 8)
    ).astype(f32)
    f8 = ml_dtypes.float8_e4m3
    WoT = Wo_p.transpose(0, 2, 1) * SCALE_WO   # [L, 256, 3008]
    wo_all = np.ascontiguousarray(
        WoT.reshape(NL, 2, P, 3008).transpose(0, 2, 1, 3)).astype(f8)
    bo_all = (bo_p * SCALE_WO).reshape(NL, 1, 3008).astype(bf)

    d_sq = float(np.mean((data_D2.astype(np.float64) ** 2).sum(-1)))
    d_mean = data_D2.mean(0).astype(f32)
    dmw = np.zeros((P, 66), f32)
    dmw[:, 0:65] = d_mean[None, :]
    dmw[:, 65] = 0.5 * d_sq - C32
    wtsr = np.tile(np.arange(F, 0, -1, dtype=f32)[None, :], (P, 1))
    ident = np.eye(P, dtype=f32)
    identb = np.eye(P, dtype=np.float32).astype(bf)
    ones1 = np.ones((1, P), bf)

    shared = dict(ident=ident, identb=identb, dmw=dmw, wtsr=wtsr, ones1=ones1,
                  wi_all=wi_all, wc_all=wc_all, b01_all=b01_all, wb_all=wb_all,
                  bb_all=bb_all, wo_all=wo_all, bo_all=bo_all)

    in_maps = []
    zf = z[:, ::-1].astype(f32) + TAIL
    for core in range(NCORES):
        s0 = core * NPC
        xs0 = np.ascontiguousarray(
            zf[s0:s0 + NPC].reshape(NTILES, P, F).transpose(1, 0, 2))
        etaT = np.ascontiguousarray(eta_batch[s0:s0 + NPC].T).astype(bf)
        m = dict(shared)
        m["xs0"] = xs0
        m["etaT"] = etaT
        in_maps.append(m)
    return in_maps


def kernel(**inputs):
    inputs = {k: np.asarray(v) for k, v in inputs.items()}
    in_maps = _host_prep(**inputs)
    if "nc" not in _CACHE:
        _CACHE["nc"] = _build_program()
    res = run_bass_kernel_spmd(_CACHE["nc"], in_maps, list(range(NCORES)))
    total = 0.0
    for r in res.results:
        total += r["loss_out"].astype(np.float64).sum()
    return np.float32(total / N_BATCH)


if __name__ == "__main__":
    pass


# revision 82
# speedup vs baseline: 1.0074x; 1.0074x over previous
"""Trainium2 Bass kernel for nn_CutBayesFlow.

Data-parallel over batch N=8192 across 8 NeuronCores (1024 samples/core).
Flow params and data summary stats are replicated; each core returns its
per-sample losses [128, 8]; the host averages to the scalar.

Spline gathers are computed as bf16 masked products + a shared 4-level
bf16 tree reduction (two monotone masks sA/sB against the bin-edge
cumsum) instead of per-bin suffix scans. x is stored pre-shifted by
+TAIL (fold into host biases) so the combine drops the -TAIL ops.

Self-contained: all shapes hardcoded, no sibling imports.
"""
import sys
import numpy as np

for _p in ("/opt/trn_rl_repo",):
    if _p not in sys.path:
        sys.path.insert(0, _p)

import ml_dtypes
import concourse.bass as bass
import concourse.bacc as bacc
import concourse.tile as tile
import concourse.mybir as mybir
from contextlib import ExitStack
from concourse.bass_utils import run_bass_kernel_spmd

F32 = mybir.dt.float32
BF16 = mybir.dt.bfloat16
FP8 = mybir.dt.float8e4
SCALE_WO = 64.0   # keep fp8 W_out out of the subnormal range
AF = mybir.ActivationFunctionType
ALU = mybir.AluOpType

# Pin all ScalarE activations to the one table that holds every function we
# use (ln, exp, relu, identity, copy, square) so the act-table-load pass
# never has to switch tables mid-kernel. Indexes are preserved; tables other
# than the chosen one are emptied so the pass cannot pick them.
_ONE_TABLE = "natural_log_exp_and_others"
_orig_get_act_tables = bacc.get_activation_tables


def _pinned_act_tables(arch):
    tabs = _orig_get_act_tables(arch)
    return {name: (fns if name == _ONE_TABLE else set())
            for name, fns in tabs.items()}


bacc.get_activation_tables = _pinned_act_tables

# ---- problem constants (hardcoded) ----
F = 64; HID = 256; P_ETA = 32; NB = 16; MULT = 3 * NB - 1   # 47
NL = 8; TAIL = 10.0
MIN_BW = 1e-8; MIN_D = 1e-8
N_BATCH = 8192; M_DATA = 256
NCORES = 8
NPC = N_BATCH // NCORES          # 1024 samples per core
P = 128                          # partitions
NTILES = NPC // P                # 8 batch-tiles per core
CHUNKS = 4                       # chunks per core
TPC = 2                          # batch-tiles per chunk
NBC = TPC * P                    # 256 samples per chunk

A_CONST = 1.0 - MIN_BW * NB
INV2TA = 1.0 / (2.0 * TAIL * A_CONST)
TWO_TA = 2.0 * TAIL * A_CONST
LOG_EPS = float(np.log(1e-10))           # -23.02585...
C32 = float(0.5 * F * np.log(2.0 * np.pi))
BOUND_D = 1.0 - MIN_D
_SCAN_ON_GPSIMD = False
_SCAN_ENG = None


def _build_program(nl=NL, debug_out=False):
    nc = bacc.Bacc("TRN2", target_bir_lowering=False, debug=False)

    # ---------------- DRAM I/O ----------------
    d_xs0 = nc.dram_tensor("xs0", [P, NTILES, F], F32, kind="ExternalInput")
    d_etaT = nc.dram_tensor("etaT", [P_ETA, NPC], BF16, kind="ExternalInput")
    d_ident = nc.dram_tensor("ident", [P, P], F32, kind="ExternalInput")
    d_identb = nc.dram_tensor("identb", [P, P], BF16, kind="ExternalInput")
    d_dmw = nc.dram_tensor("dmw", [P, 66], F32, kind="ExternalInput")
    d_wtsr = nc.dram_tensor("wtsr", [P, F], F32, kind="ExternalInput")
    d_ones = nc.dram_tensor("ones1", [1, P], BF16, kind="ExternalInput")
    d_wi = nc.dram_tensor("wi_all", [NL, F, HID], BF16, kind="ExternalInput")
    d_wc = nc.dram_tensor("wc_all", [NL, P_ETA, HID], BF16, kind="ExternalInput")
    d_b01 = nc.dram_tensor("b01_all", [NL, P, 2], F32, kind="ExternalInput")
    d_wb = nc.dram_tensor("wb_all", [NL, P, 2, 2, 2, 2, P], BF16,
                          kind="ExternalInput")
    d_bb = nc.dram_tensor("bb_all", [NL, P, 8], F32, kind="ExternalInput")
    d_wo = nc.dram_tensor("wo_all", [NL, P, 2, 3008], FP8, kind="ExternalInput")
    d_bo = nc.dram_tensor("bo_all", [NL, 1, 3008], BF16, kind="ExternalInput")
    d_loss = nc.dram_tensor("loss_out", [P, NTILES], F32, kind="ExternalOutput")
    d_dbg = None
    if debug_out:
        d_dbg = nc.dram_tensor("dbg_out", [P, 2 * NTILES, F], F32,
                               kind="ExternalOutput")

    bctx = ExitStack()
    with tile.TileContext(nc) as tc:
        with bctx:
            _emit(bctx, tc, nc, d_xs0, d_etaT, d_ident, d_identb, d_dmw, d_wtsr,
                  d_ones, d_wi, d_wc, d_b01, d_wb, d_bb, d_wo, d_bo, d_loss,
                  nl=nl, d_dbg=d_dbg)
    nc.compile()
    return nc


def _fv(buf, off, dims):
    """View of a tile at element offset `off` with [stride, count] dims."""
    a = buf[:]
    return bass.AP(a.tensor, a.offset + off, [list(a.ap[0])] + dims)


def _emit(ctx, tc, nc, d_xs0, d_etaT, d_ident, d_identb, d_dmw, d_wtsr, d_ones,
          d_wi, d_wc, d_b01, d_wb, d_bb, d_wo, d_bo, d_loss, nl=NL, d_dbg=None):
    CB = TPC * F * NB             # 2048 per-partition bin elems per chunk
    Z = TPC * F                   # 128 combine width per chunk
    NSEG = TPC * F                # 128 bin segments per chunk

    global _SCAN_ENG
    _SCAN_ENG = nc.gpsimd if _SCAN_ON_GPSIMD else nc.vector
    pconst = ctx.enter_context(tc.tile_pool(name="pconst", bufs=1))
    pw = ctx.enter_context(tc.tile_pool(name="pw", bufs=2))
    pmlp = ctx.enter_context(tc.tile_pool(name="pmlp", bufs=2))
    pcmb = ctx.enter_context(tc.tile_pool(name="pcmb", bufs=1))
    ps_mm = ctx.enter_context(tc.tile_pool(name="ps_mm", bufs=2, space="PSUM"))
    ps_uw = ctx.enter_context(tc.tile_pool(name="ps_uw", bufs=1, space="PSUM"))
    ps_uh = ctx.enter_context(tc.tile_pool(name="ps_uh", bufs=1, space="PSUM"))
    ps_ud = ctx.enter_context(tc.tile_pool(name="ps_ud", bufs=1, space="PSUM"))

    # ---------------- persistent tiles ----------------
    xs = pconst.tile([P, NTILES, F], F32, tag="xs")     # holds x + TAIL
    nc.sync.dma_start(xs[:], d_xs0[:])
    etaT = pconst.tile([P_ETA, NPC], BF16, tag="etaT")
    nc.sync.dma_start(etaT[:], d_etaT[:])
    ident = pconst.tile([P, P], F32, tag="ident")
    nc.sync.dma_start(ident[:], d_ident[:])
    identb = pconst.tile([P, P], BF16, tag="identb")
    nc.sync.dma_start(identb[:], d_identb[:])
    dmw = pconst.tile([P, 66], F32, tag="dmw")
    nc.sync.dma_start(dmw[:], d_dmw[:])
    wtsr = pconst.tile([P, F], F32, tag="wtsr")
    nc.sync.dma_start(wtsr[:], d_wtsr[:])
    ones1 = pconst.tile([1, P], BF16, tag="ones1")
    nc.sync.dma_start(ones1[:], d_ones[:])

    segm = pconst.tile([P, CB], BF16, tag="segm")     # 1 except 0 at k=0 of 16
    nc.vector.memset(segm[:], 1.0)
    nc.vector.memset(segm[:, 0::NB], 0.0)
    # per-bin sources packed in one mega buffer [ew | eh | dbR | dbL], each
    # lane flat [P, 2048] bf16, logical layout (t, f, k). Double-buffered so
    # chunk c+1's ScalarE evacs don't stall on chunk c's DVE reads.
    binsrc2 = []
    for i in range(2):
        bsi = pconst.tile([P, 4 * CB], BF16, tag=f"bs{i}", name=f"bsi{i}")
        nc.vector.memset(bsi[:, 2 * CB::NB], BOUND_D)        # dbR slot0 = d[0]
        nc.vector.memset(bsi[:, 3 * CB + NB - 1::NB], BOUND_D)  # dbL slot15
        binsrc2.append(bsi)
    E_t2 = [pconst.tile([P, CB], F32, tag=f"E_t{i}", name=f"E_t{i}")
            for i in range(2)]                        # incl cumsum of ew (exact)
    sB = pconst.tile([P, CB], BF16, tag="sB")         # [X >= E_k]
    ind = pconst.tile([P, CB], BF16, tag="ind")       # one-hot bin indicator
    # products scratch: 4 exact-selection lanes [ind*ew, ind*eh, ind*dbR,
    # ind*dbL] (bf16 tree is exact: zeros + one value) + tree lanes 4/5 for
    # the cumsum gPp (sB*eh) and total Sh (copy of eh); lane 6 (sB*ew) is
    # reduced with f32 accumulation (exact) because u = Xt - gP cancels.
    scrP = pconst.tile([P, 6 * CB], BF16, tag="scrP")
    scrA = pconst.tile([P, 4 * CB // 2], BF16, tag="scrA")
    res6 = pconst.tile([P, 4 * NSEG], F32, tag="res6")
    gPx = pconst.tile([P, 2 * NSEG], F32, tag="gPx")
    ld_t = pconst.tile([P, NTILES], F32, tag="ld_t")
    ldacc = pconst.tile([P, NTILES, F], F32, tag="ldacc")
    nc.vector.memset(ldacc[:], 0.0)
    zz_t = pconst.tile([P, NTILES], F32, tag="zz_t")
    negT = pconst.tile([P, 1], F32, tag="negT")
    nc.vector.memset(negT[:], -TAIL)

    def seg3(buf, off, n=NB):
        # [P, NSEG, n] view of a flat [P, CB]-ish buffer at element offset
        return _fv(buf, off, [[NB, NSEG], [1, n]])

    def colNB(buf, c):
        # [P, TPC, F] view picking bin-column c
        return _fv(buf, c, [[F * NB, TPC], [NB, F]])

    def v3(buf, off=0):
        # [P, TPC, F] view of a flat [P, Z] f32 tile
        return _fv(buf, off, [[F, TPC], [1, F]])

    pending = {}

    def flush_pending():
        if not pending:
            return
        inner, vv, ldv = pending.pop("inner"), pending.pop("vv"), pending.pop("ldv")
        lni = pcmb.tile([P, Z], F32, tag="lni", name="lni")
        lnv = pcmb.tile([P, Z], F32, tag="lnv", name="lnv")
        nc.scalar.activation(lni[:], inner[:], AF.Ln)
        nc.scalar.activation(lnv[:], vv[:], AF.Ln)
        ldf = pcmb.tile([P, Z], F32, tag="ldf", name="ldf")
        nc.vector.scalar_tensor_tensor(ldf[:], lnv[:], 2.0, lni[:], ALU.mult,
                                       ALU.add)
        nc.vector.tensor_tensor(ldv, ldv, ldf[:].rearrange("p (t f) -> p t f", t=TPC),
                                ALU.add)

    # ---------------- layer loop ----------------
    for l in range(nl):
        wi = pw.tile([F, HID], BF16, tag="wi")
        nc.sync.dma_start(wi[:], d_wi[l])
        wc = pw.tile([P_ETA, HID], BF16, tag="wc")
        nc.sync.dma_start(wc[:], d_wc[l])
        b01 = pw.tile([P, 2], F32, tag="b01")
        nc.sync.dma_start(b01[:], d_b01[l])
        wb = pw.tile([P, 2, 2, 2, 2, P], BF16, tag="wb")
        nc.sync.dma_start(wb[:], d_wb[l])
        bb = pw.tile([P, 8], F32, tag="bb")
        nc.sync.dma_start(bb[:], d_bb[l])
        wo = pw.tile([P, 2, 3008], FP8, tag="wo")
        nc.sync.dma_start(wo[:], d_wo[l])
        bo = pw.tile([1, 3008], BF16, tag="bo")
        nc.sync.dma_start(bo[:], d_bo[l])

        for c in range(CHUNKS):
            bs = binsrc2[c % 2]
            # ---- x transpose -> xT bf16 [F, NBC] (xT carries x + TAIL) ----
            xT = pmlp.tile([F, NBC], BF16, tag="xT")
            for t in range(TPC):
                pt = ps_mm.tile([F, P], F32, tag="mm")
                nc.tensor.matmul(pt[:], xs[:, TPC * c + t, :], ident[:],
                                 is_transpose=True)
                nc.scalar.copy(xT[:, t * P:(t + 1) * P], pt[:])
                if l == 0:
                    scr = pcmb.tile([P, F], F32, tag="sqscr")
                    nc.scalar.activation(scr[:], xs[:, TPC * c + t, :], AF.Square,
                                         bias=negT[:],
                                         accum_out=zz_t[:, TPC * c + t:TPC * c + t + 1])
            # ---- h = x@WiT + eta@WcT (+bias; TAIL-shift folded into b01).
            # No DVE ops anywhere in the MLP: the DVE instruction queue stays
            # pure-spline, so chunk c+1's MLP pipelines under chunk c's DVE.
            h_sb = pmlp.tile([P, 2, HID], BF16, tag="h")
            r = pmlp.tile([P, 2, HID], BF16, tag="r")
            eta_sl = etaT[:, c * NBC:(c + 1) * NBC]
            phs = []
            for mc in range(2):
                ph = ps_mm.tile([P, NBC], F32, tag="mm")
                nc.tensor.matmul(ph[:], wi[:, mc * P:(mc + 1) * P], xT[:],
                                 start=True, stop=False)
                nc.tensor.matmul(ph[:], wc[:, mc * P:(mc + 1) * P], eta_sl,
                                 start=False, stop=True)
                phs.append(ph)
            for mc in range(2):
                nc.scalar.activation(h_sb[:, mc], phs[mc][:], AF.Identity,
                                     bias=b01[:, mc:mc + 1])
            for mc in range(2):
                nc.scalar.activation(r[:, mc], phs[mc][:], AF.Relu,
                                     bias=b01[:, mc:mc + 1])
            # ---- residual blocks (residual added via identity-matmul into
            # PSUM so h_new/r_nxt come out of ScalarE evacs, not DVE) ----
            for blk in range(2):
                r1 = pmlp.tile([P, 2, HID], BF16, tag="r1")
                for mc in range(2):
                    pt1 = ps_mm.tile([P, NBC], F32, tag="mm")
                    for kc in range(2):
                        nc.tensor.matmul(pt1[:], wb[:, blk, 0, kc, mc, :],
                                         r[:, kc], start=(kc == 0), stop=(kc == 1))
                    j = blk * 4 + 0 * 2 + mc
                    nc.scalar.activation(r1[:, mc], pt1[:], AF.Relu,
                                         bias=bb[:, j:j + 1])
                h_new = pmlp.tile([P, 2, HID], BF16 if blk == 0 else FP8,
                                  tag="h" if blk == 0 else "h8", name="h_new")
                r_nxt = pmlp.tile([P, 2, HID], BF16, tag="r")
                pt2s = []
                for mc in range(2):
                    pt2 = ps_mm.tile([P, NBC], F32, tag="mm")
                    nc.tensor.matmul(pt2[:], identb[:], h_sb[:, mc],
                                     start=True, stop=False)
                    for kc in range(2):
                        nc.tensor.matmul(pt2[:], wb[:, blk, 1, kc, mc, :],
                                         r1[:, kc], start=False, stop=(kc == 1))
                    pt2s.append(pt2)
                for mc in range(2):
                    j = blk * 4 + 1 * 2 + mc
                    nc.scalar.activation(h_new[:, mc], pt2s[mc][:], AF.Identity,
                                         bias=bb[:, j:j + 1])
                if blk == 0:
                    for mc in range(2):
                        j = blk * 4 + 1 * 2 + mc
                        nc.scalar.activation(r_nxt[:, mc], pt2s[mc][:], AF.Relu,
                                             bias=bb[:, j:j + 1])
                h_sb = h_new
                r = r_nxt
            # ---- out matmuls + Exp/Softplus evacs per batch-tile ----
            for t in range(TPC):
                p_uw = ps_uw.tile([P, 1024], F32, tag="puw")
                p_uh = ps_uh.tile([P, 1024], F32, tag="puh")
                p_ud = ps_ud.tile([P, 960], F32, tag="pud")
                for (ps_t, cstart, total) in ((p_uw, 0, 1024), (p_uh, 1024, 1024),
                                              (p_ud, 2048, 960)):
                    n0 = 0
                    while n0 < total:
                        nsz = min(512, total - n0)
                        sl = ps_t[:, n0:n0 + nsz]
                        nc.tensor.matmul(sl, ones1[:],
                                         bo[:, cstart + n0:cstart + n0 + nsz],
                                         start=True, stop=False)
                        nc.tensor.matmul(sl, h_sb[:, :, t * P:(t + 1) * P],
                                         wo[:, :, cstart + n0:cstart + n0 + nsz],
                                         start=False, stop=True,
                                         perf_mode=mybir.MatmulPerfMode.DoubleRow)
                        n0 += nsz
                nc.scalar.activation(_fv(bs, 0 * CB + t * F * NB, [[1, 1024]]),
                                     p_uw[:], AF.Exp, scale=1.0 / 16.0 / SCALE_WO)
                nc.scalar.activation(_fv(bs, 1 * CB + t * F * NB, [[1, 1024]]),
                                     p_uh[:], AF.Exp, scale=1.0 / 16.0 / SCALE_WO)
                spe = pcmb.tile([P, 960], BF16, tag=f"spe{t}", name=f"spe{t}")
                nc.scalar.activation(spe[:], p_ud[:], AF.Exp, scale=1.0 / SCALE_WO)
                nc.scalar.activation(_fv(bs, 2 * CB + t * F * NB + 1,
                                         [[NB, F], [1, NB - 1]]),
                                     spe[:], AF.Ln, bias=1.0)
                nc.scalar.activation(_fv(bs, 3 * CB + t * F * NB,
                                         [[NB, F], [1, NB - 1]]),
                                     spe[:], AF.Ln, bias=1.0)
            flush_pending()
            # ---- spline section (cumsum scan on GpSimd, rest on DVE) ----
            E_t = E_t2[c % 2]
            _SCAN_ENG.tensor_tensor_scan(E_t[:], segm[:], _fv(bs, 0, [[1, CB]]),
                                         0.0, ALU.mult, ALU.add)
            # Xt = (x + T) * S_w * INV2TA  (xs holds x+T)
            Xt = pcmb.tile([P, Z], F32, tag="Xt")
            xs_ch = xs[:, TPC * c:TPC * (c + 1), :]
            nc.vector.scalar_tensor_tensor(v3(Xt), xs_ch, INV2TA, colNB(E_t, NB - 1),
                                           ALU.mult, ALU.mult)
            # mask vs exact f32 edges: sB_k = [X >= E_k] (k=0..15)
            Xb16 = _fv(Xt, 0, [[1, Z], [0, NB]])
            nc.vector.tensor_tensor(seg3(sB, 0), Xb16, seg3(E_t, 0), ALU.is_ge)
            # one-hot indicator: ind_k = sB_{k-1} - sB_k (sB_{-1} := 1)
            nc.vector.tensor_tensor(seg3(ind, 1, NB - 1), seg3(sB, 0, NB - 1),
                                    seg3(sB, 1, NB - 1), ALU.subtract)
            nc.vector.tensor_scalar(_fv(ind, 0, [[NB, NSEG]]),
                                    _fv(sB, 0, [[NB, NSEG]]),
                                    -1.0, 1.0, ALU.mult, ALU.add)
            # products: lanes 0-3 ind-masked, lane 4 sB*ew, lane 5 sB*eh
            # (flat [P, 2048] bf16, 2x mode)
            for q, (soff, msk) in enumerate(((0, ind), (CB, ind), (2 * CB, ind),
                                             (3 * CB, ind), (0, sB), (CB, sB))):
                nc.vector.tensor_tensor(_fv(scrP, q * CB, [[1, CB]]),
                                        _fv(bs, soff, [[1, CB]]),
                                        msk[:], ALU.mult)
            # 4-level exact bf16 tree over lanes 0-3 -> res6 [P, 4*NSEG] f32
            NL1 = 4 * NSEG  # 512 segments across selection lanes
            nc.vector.tensor_tensor(_fv(scrA, 0, [[8, NL1], [1, 8]]),
                                    _fv(scrP, 0, [[NB, NL1], [1, 8]]),
                                    _fv(scrP, 8, [[NB, NL1], [1, 8]]), ALU.add)
            nc.vector.tensor_tensor(_fv(scrP, 0, [[4, NL1], [1, 4]]),
                                    _fv(scrA, 0, [[8, NL1], [1, 4]]),
                                    _fv(scrA, 4, [[8, NL1], [1, 4]]), ALU.add)
            nc.vector.tensor_tensor(_fv(scrA, 0, [[2, NL1], [1, 2]]),
                                    _fv(scrP, 0, [[4, NL1], [1, 2]]),
                                    _fv(scrP, 2, [[4, NL1], [1, 2]]), ALU.add)
            nc.vector.tensor_tensor(_fv(res6, 0, [[1, NL1]]),
                                    _fv(scrA, 0, [[2, NL1]]),
                                    _fv(scrA, 1, [[2, NL1]]), ALU.add)
            # lanes 4/5 (sB*ew, sB*eh): one f32-accumulated segment reduce ->
            # exact exclusive cumsums at idx (consistent with the f32 scan)
            nc.vector.tensor_reduce(_fv(gPx, 0, [[1, 2 * NSEG]]),
                                    _fv(scrP, 4 * CB, [[NB, 2 * NSEG], [1, NB]]),
                                    mybir.AxisListType.X, ALU.add)
            # Sh = sum_k eh_k
            Sh_t = pcmb.tile([P, Z], F32, tag="Sh_t", name="Sh_t")
            nc.vector.tensor_reduce(v3(Sh_t), _fv(bs, CB, [[NB, NSEG], [1, NB]]),
                                    mybir.AxisListType.X, ALU.add)
            # ---- combine ([P, Z]) ----
            Qe_s = _fv(res6, 0 * Z, [[1, Z]]); Qh_s = _fv(res6, 1 * Z, [[1, Z]])
            d0 = _fv(res6, 2 * Z, [[1, Z]]); d1 = _fv(res6, 3 * Z, [[1, Z]])
            gPp_v = _fv(gPx, NSEG, [[1, Z]])
            Sh_v = Sh_t[:]
            cZ = lambda tg: pcmb.tile([P, Z], F32, tag=tg, name=tg)
            TT = nc.vector.tensor_tensor
            TS = nc.vector.tensor_scalar
            STT = nc.vector.scalar_tensor_tensor
            gTT = nc.gpsimd.tensor_tensor
            Qe = cZ("Qe"); nc.vector.tensor_scalar_max(Qe[:], Qe_s, 1e-6)
            u = cZ("u"); TT(u[:], Xt[:], _fv(gPx, 0, [[1, Z]]), ALU.subtract)
            iQe = cZ("iQe"); nc.vector.reciprocal_approx_fast(iQe[:], Qe[:])
            tv = cZ("tv"); TT(tv[:], u[:], iQe[:], ALU.mult)
            om = cZ("om"); TS(om[:], tv[:], -1.0, 1.0, ALU.mult, ALU.add)
            ttv = cZ("ttv"); TT(ttv[:], tv[:], om[:], ALU.mult)
            rSh = cZ("rSh"); nc.vector.reciprocal_approx_fast(rSh[:], Sh_v)
            rho = cZ("rho"); nc.vector.tensor_scalar_mul(rho[:], rSh[:], TWO_TA)
            q_ = cZ("q"); TT(q_[:], Qh_s, iQe[:], ALU.mult)
            sr = cZ("sr"); TT(v3(sr), colNB(E_t, NB - 1), v3(rSh), ALU.mult)
            delta = cZ("delta"); TT(delta[:], q_[:], sr[:], ALU.mult)
            a1 = cZ("a1"); gTT(a1[:], d0, d1, ALU.add)
            a2 = cZ("a2"); STT(a2[:], delta[:], -2.0, a1[:], ALU.mult, ALU.add)
            den = cZ("den"); TT(den[:], a2[:], ttv[:], ALU.mult)
            TT(den[:], den[:], delta[:], ALU.add)
            idn = cZ("idn"); nc.vector.reciprocal_approx_fast(idn[:], den[:])
            t2 = cZ("t2"); gTT(t2[:], tv[:], tv[:], ALU.mult)
            om2 = cZ("om2"); gTT(om2[:], om[:], om[:], ALU.mult)
            b1 = cZ("b1"); gTT(b1[:], d1, t2[:], ALU.mult)
            b2 = cZ("b2"); gTT(b2[:], delta[:], ttv[:], ALU.mult)
            b3 = cZ("b3"); gTT(b3[:], d0, om2[:], ALU.mult)
            inner = cZ("inner"); STT(inner[:], b2[:], 2.0, b1[:], ALU.mult, ALU.add)
            TT(inner[:], inner[:], b3[:], ALU.add)
            n1 = cZ("n1"); gTT(n1[:], delta[:], t2[:], ALU.mult)
            n2 = cZ("n2"); gTT(n2[:], d0, ttv[:], ALU.mult)
            numy = cZ("numy"); gTT(numy[:], n1[:], n2[:], ALU.add)
            ich = cZ("ich"); TT(ich[:], rho[:], gPp_v, ALU.mult)
            ih = cZ("ih"); TT(ih[:], rho[:], Qh_s, ALU.mult)
            g_ = cZ("g_"); TT(g_[:], numy[:], idn[:], ALU.mult)
            yv = cZ("yv"); TT(yv[:], ih[:], g_[:], ALU.mult)
            # write y+TAIL directly into xs via the final add (a compute op --
            # must not be copy-propagated into the next layer's transpose)
            TT(xs_ch, v3(yv), v3(ich), ALU.add)
            vv = cZ("vv"); gTT(vv[:], delta[:], idn[:], ALU.mult)
            pending["inner"] = inner
            pending["vv"] = vv
            pending["ldv"] = ldacc[:, TPC * c:TPC * (c + 1), :]

    flush_pending()
    nc.vector.tensor_reduce(ld_t[:], ldacc[:], mybir.AxisListType.X, ALU.add)

    if d_dbg is not None:
        # dump xs (x+TAIL after `nl` layers) and per-sample flow ld
        nc.sync.dma_start(d_dbg[:, 0:NTILES, :], xs[:])
        scrd = pcmb.tile([P, NTILES, F], F32, tag="scrd")
        nc.vector.memset(scrd[:], 0.0)
        nc.vector.tensor_copy(scrd[:, :, 0], ld_t[:])
        nc.vector.tensor_copy(scrd[:, :, 1], zz_t[:])
        nc.sync.dma_start(d_dbg[:, NTILES:2 * NTILES, :], scrd[:])

    # ---------------- tail: stick-breaking + loss ----------------
    SPN = NTILES * (F + 1)        # 520

    def tb(tag, n):
        return pconst.tile([P, n], F32, tag=tag, name=tag)

    # stored x+T after 8 layers is feature-reversed (parity of last layer=1);
    # un-reverse and un-shift (-TAIL) in one DVE op.
    xr_rev = bass.AP(xs[:].tensor, xs[:].offset + (F - 1),
                     [list(xs[:].ap[0]), [F, NTILES], [-1, F]])
    xnat = pconst.tile([P, NTILES, F], F32, tag="xnat")
    nc.vector.tensor_scalar(xnat[:], xr_rev, 1.0, -TAIL, ALU.mult, ALU.add)
    xr = xnat[:]
    t_spp = tb("t_spp", SPN)
    spp = _fv(t_spp, 0, [[F + 1, NTILES], [1, F + 1]])
    nc.vector.memset(spp[:, :, 0], 0.0)
    t_spn = tb("t_spn", NTILES * F)
    spn = _fv(t_spn, 0, [[F, NTILES], [1, F]])
    t_e1 = tb("t_e1", NTILES * F)
    e1 = _fv(t_e1, 0, [[F, NTILES], [1, F]])
    nc.scalar.activation(e1, xr, AF.Exp)
    nc.scalar.activation(spp[:, :, 1:F + 1], e1, AF.Ln, bias=1.0)
    nc.scalar.activation(e1, xr, AF.Exp, scale=-1.0)
    nc.scalar.activation(spn, e1, AF.Ln, bias=1.0)
    t_seg = tb("t_seg", SPN)
    nc.vector.memset(t_seg[:], 1.0)
    nc.vector.memset(t_seg[:, 0::F + 1], 0.0)
    t_cum = tb("t_cum", SPN)
    nc.vector.tensor_tensor_scan(t_cum[:], t_seg[:], t_spp[:], 0.0,
                                 ALU.mult, ALU.add)
    cum3 = _fv(t_cum, 0, [[F + 1, NTILES], [1, F + 1]])
    t_ltn = tb("t_ltn", SPN)
    ltn = _fv(t_ltn, 0, [[F + 1, NTILES], [1, F + 1]])
    nc.vector.tensor_tensor(ltn[:, :, 0:F], spn, cum3[:, :, 0:F], ALU.add)
    nc.vector.tensor_copy(ltn[:, :, F], cum3[:, :, F])
    t_mn = tb("t_mn", SPN)
    mn = _fv(t_mn, 0, [[F + 1, NTILES], [1, F + 1]])
    nc.vector.tensor_scalar_min(mn, ltn, -LOG_EPS)
    lpn = pconst.tile([P, NTILES], F32, tag="lpn")
    nc.vector.tensor_reduce(lpn[:], mn, mybir.AxisListType.X, ALU.add)
    t_th = tb("t_th", SPN)
    th = _fv(t_th, 0, [[F + 1, NTILES], [1, F + 1]])
    nc.scalar.activation(th, ltn, AF.Exp, scale=-1.0)
    t_q1 = tb("t_q1", SPN)
    q1 = _fv(t_q1, 0, [[F + 1, NTILES], [1, F + 1]])
    dmb = dmw[:, 0:F + 1].unsqueeze(1).broadcast_to([P, NTILES, F + 1])
    nc.vector.scalar_tensor_tensor(q1, dmb, -2.0, th, ALU.mult, ALU.add)
    nc.vector.tensor_tensor(q1, q1, th, ALU.mult)
    qs = pconst.tile([P, NTILES], F32, tag="qs")
    nc.vector.tensor_reduce(qs[:], q1, mybir.AxisListType.X, ALU.add)
    t_w1 = tb("t_w1", NTILES * F)
    w1 = _fv(t_w1, 0, [[F, NTILES], [1, F]])
    wtb = wtsr[:].unsqueeze(1).broadcast_to([P, NTILES, F])
    nc.vector.tensor_tensor(w1, wtb, spp[:, :, 1:F + 1], ALU.mult)
    nc.vector.tensor_tensor(w1, w1, spn, ALU.add)
    sbn = pconst.tile([P, NTILES], F32, tag="sbn")
    nc.vector.tensor_reduce(sbn[:], w1, mybir.AxisListType.X, ALU.add)
    # loss = -0.5 zz - C32 - ld + sbn + lpn + 0.5 qs + (0.5 d_sq)
    o1 = pconst.tile([P, NTILES], F32, tag="o1")
    nc.vector.scalar_tensor_tensor(o1[:], zz_t[:], -0.5, ld_t[:], ALU.mult,
                                   ALU.subtract)
    o2 = pconst.tile([P, NTILES], F32, tag="o2")
    nc.vector.scalar_tensor_tensor(o2[:], qs[:], 0.5, sbn[:], ALU.mult, ALU.add)
    nc.vector.tensor_tensor(o1[:], o1[:], o2[:], ALU.add)
    nc.vector.tensor_tensor(o1[:], o1[:], lpn[:], ALU.add)
    loss = pconst.tile([P, NTILES], F32, tag="loss")
    nc.vector.tensor_scalar_add(loss[:], o1[:], dmw[:, 65:66])
    nc.sync.dma_start(d_loss[:], loss[:])


# ---------------- host side ----------------
_CACHE = {}


def _host_prep(z, eta_batch, data_D2, W_in, b_in, W_ctx, b_ctx, W_blk, b_blk,
               W_out, b_out):
    f32 = np.float32
    bf = ml_dtypes.bfloat16
    in_deg = np.arange(1, F + 1)
    hid_deg = np.arange(HID) % (F - 1) + 1
    m_in = (hid_deg[:, None] >= in_deg[None, :]).astype(f32)
    m_hh = (hid_deg[:, None] >= hid_deg[None, :]).astype(f32)
    out_deg = np.repeat(in_deg, MULT)
    m_out = (out_deg[:, None] > hid_deg[None, :]).astype(f32)

    Wi = (W_in * m_in[None]).astype(f32)
    Wb = (W_blk * m_hh[None, None, None]).astype(f32)
    Wo_m = (W_out * m_out[None]).astype(f32)
    # fold the per-layer feature flip into host permutations (parity trick):
    # stored feature j corresponds to logical feature lj = F-1-j on odd layers
    Wi_eff = np.empty_like(Wi)
    Wo_p = np.empty_like(Wo_m)
    bo_p = np.empty_like(b_out)
    for l in range(NL):
        par = l % 2
        Wi_eff[l] = Wi[l][:, ::-1] if par else Wi[l]
        perm = np.empty(3008, dtype=np.int64)
        for j in range(F):
            lj = (F - 1 - j) if par else j
            perm[j * 16:(j + 1) * 16] = lj * 47 + np.arange(16)
            perm[1024 + j * 16:1024 + (j + 1) * 16] = lj * 47 + 16 + np.arange(16)
            perm[2048 + j * 15:2048 + (j + 1) * 15] = lj * 47 + 32 + np.arange(15)
        Wo_p[l] = Wo_m[l][perm, :]
        bo_p[l] = b_out[l][perm]
    bo_p = bo_p.astype(f32)

    wi_all = np.ascontiguousarray(Wi_eff.transpose(0, 2, 1)).astype(bf)  # [L,64,256]
    wc_all = np.ascontiguousarray(W_ctx.transpose(0, 2, 1)).astype(bf)   # [L,32,256]
    # xs carries x+TAIL: fold -TAIL * rowsum(Wi_eff) into the bias
    wi_rowsum = np.asarray(wi_all, dtype=f32).sum(axis=1)                # [L, HID]
    b01 = (b_in + b_ctx - TAIL * wi_rowsum).astype(f32)
    b01_all = np.ascontiguousarray(
        b01.reshape(NL, 2, P).transpose(0, 2, 1)).astype(f32)
    # wb_all[l, p, blk, sub, kc, mc, m] = (Wb[l,blk,sub]*m_hh).T[kc*128+p, mc*128+m]
    WbT = Wb.transpose(0, 1, 2, 4, 3)          # [L, blk, sub, in(256), out(256)]
    wb6 = WbT.reshape(NL, 2, 2, 2, P, 2, P)    # [L, blk, sub, kc, p, mc, m]
    wb_all = np.ascontiguousarray(
        wb6.transpose(0, 4, 1, 2, 3, 5, 6)).astype(bf)  # [L, p, blk, sub, kc, mc, m]
    bb_all = np.ascontiguousarray(
        b_blk.reshape(NL, 2, 2, 2, P).transpose(0, 4, 1, 2, 3).reshape(NL, P, 8)
    ).astype(f32)
    f8 = ml_dtypes.float8_e4m3
    WoT = Wo_p.transpose(0, 2, 1) * SCALE_WO   # [L, 256, 3008]
    wo_all = np.ascontiguousarray(
        WoT.reshape(NL, 2, P, 3008).transpose(0, 2, 1, 3)).astype(f8)
    bo_all = (bo_p * SCALE_WO).reshape(NL, 1, 3008).astype(bf)

    d_sq = float(np.mean((data_D2.astype(np.float64) ** 2).sum(-1)))
    d_mean = data_D2.mean(0).astype(f32)
    dmw = np.zeros((P, 66), f32)
    dmw[:, 0:65] = d_mean[None, :]
    dmw[:, 65] = 0.5 * d_sq - C32
    wtsr = np.tile(np.arange(F, 0, -1, dtype=f32)[None, :], (P, 1))
    ident = np.eye(P, dtype=f32)
    identb = np.eye(P, dtype=np.float32).astype(bf)
    ones1 = np.ones((1, P), bf)

    shared = dict(ident=ident, identb=identb, dmw=dmw, wtsr=wtsr, ones1=ones1,
                  wi_all=wi_all, wc_all=wc_all, b01_all=b01_all, wb_all=wb_all,
                  bb_all=bb_all, wo_all=wo_all, bo_all=bo_all)

    in_maps = []
    zf = z[:, ::-1].astype(f32) + TAIL
    for core in range(NCORES):
        s0 = core * NPC
        xs0 = np.ascontiguousarray(
            zf[s0:s0 + NPC].reshape(NTILES, P, F).transpose(1, 0, 2))
        etaT = np.ascontiguousarray(eta_batch[s0:s0 + NPC].T).astype(bf)
        m = dict(shared)
        m["xs0"] = xs0
        m["etaT"] = etaT
        in_maps.append(m)
    return in_maps


def kernel(**inputs):
    inputs = {k: np.asarray(v) for k, v in inputs.items()}
    in_maps = _host_prep(**inputs)
    if "nc" not in _CACHE:
        _CACHE["nc"] = _build_program()
    res = run_bass_kernel_spmd(_CACHE["nc"], in_maps, list(range(NCORES)))
    total = 0.0
    for r in res.results:
        total += r["loss_out"].astype(np.float64).sum()
    return np.float32(total / N_BATCH)


if __name__ == "__main__":
    pass
